# revision 4
# baseline (speedup 1.0000x reference)
"""Supervised-contrastive loss on 8 TRN2 NeuronCores.

Math (matches the reference exactly):
    s_ij   = cosine similarity of feature rows i, j
    E_ij   = exp(s_ij / tau)
    neg_i  = sum_j E_ij * (1 - mask_ij)        (mask = same-class, incl. diag)
    loss   = sum over i and same-class j != i of [ln(E_ij + neg_i) - s_ij/tau] / p_i
             ------------------------------------------------------------
                                  sum_i p_i

Device (per core, rows r in [c*512, (c+1)*512)):
    GEMM S = lhsT.T @ fnT (bf16, f32 PSUM), E = exp(S/tau) on ACT with fused
    row-accumulate, mask via tensor_scalar(is_equal), neg via fused
    tensor_tensor_reduce, then A_i = sum_j mask * ln(E + neg_i) with the ln
    bias doing the "+neg_i" and a second tensor_tensor_reduce doing the
    masked row sum.  Outputs per-row A_i and neg_i.

Host (O(N*D) prep/postprocess only):
    row normalization, the linear term B_i = fn_i . g(t_i) / tau via class
    sums, the diagonal-pair correction ln(e^{1/tau} + neg_i) - 1/tau, and the
    final scalar reduction  loss = sum((A - B - corr)/p) / sum(p).
"""

import numpy as np
import ml_dtypes

TAU = 0.1
N, D = 4096, 512
NCORES = 8
ROWS = N // NCORES          # 512 rows per core
ITILES = ROWS // 128        # 4 partition tiles per core
CH = N // 512               # 8 column chunks of 512
KT = D // 128               # 4 contraction tiles

_CACHE = {}


def _build_nc():
    import concourse.tile as tile
    import concourse.mybir as mybir
    from concourse import bacc

    dt = mybir.dt
    AF = mybir.ActivationFunctionType
    ALU = mybir.AluOpType
    AX = mybir.AxisListType

    nc = bacc.Bacc(None)
    fnT = nc.declare_dram_parameter("fnT", [D, N], dt.bfloat16, isOutput=False)
    lhsT = nc.declare_dram_parameter("lhsT", [D, ROWS], dt.bfloat16, isOutput=False)
    tb = nc.declare_dram_parameter("tb", [128, N], dt.bfloat16, isOutput=False)
    tcol = nc.declare_dram_parameter("tcol", [128, ITILES], dt.float32, isOutput=False)
    a_out = nc.declare_dram_parameter("a_out", [128, ITILES], dt.float32, isOutput=True)
    neg_out = nc.declare_dram_parameter("neg_out", [128, ITILES], dt.float32, isOutput=True)

    with tile.TileContext(nc) as tc:
        with (
            tc.tile_pool(name="persist", bufs=1) as persist,
            tc.tile_pool(name="psum", bufs=8, space="PSUM") as psum,
            tc.tile_pool(name="ebuf", bufs=16) as ebuf,
            tc.tile_pool(name="lbuf", bufs=4) as lbuf,
            tc.tile_pool(name="scr", bufs=4) as scr,
            tc.tile_pool(name="acc", bufs=2) as accp,
            tc.tile_pool(name="outp", bufs=1) as outp,
        ):
            # ---- persistent loads (quarters so matmuls can start early) ----
            fn_sb = []  # [kt][quarter] -> [128, 1024]
            for k in range(KT):
                row0 = k * 128
                qs = []
                for q in range(4):
                    tq = persist.tile([128, 1024], dt.bfloat16, tag=f"fnt_{k}_{q}")
                    nc.sync.dma_start(
                        tq[:], fnT[row0:row0 + 128, q * 1024:(q + 1) * 1024]
                    )
                    qs.append(tq)
                fn_sb.append(qs)
            lhs_sb = []
            for k in range(KT):
                tk = persist.tile([128, ROWS], dt.bfloat16, tag=f"lhs_{k}")
                nc.sync.dma_start(tk[:], lhsT[k * 128:(k + 1) * 128, :])
                lhs_sb.append(tk)
            tb_sb = persist.tile([128, N], dt.bfloat16, tag="tb")
            nc.sync.dma_start(tb_sb[:], tb[:])
            tcol_sb = persist.tile([128, ITILES], dt.float32, tag="tcol")
            nc.sync.dma_start(tcol_sb[:], tcol[:])

            aout_sb = outp.tile([128, ITILES], dt.float32, tag="aout")
            negout_sb = outp.tile([128, ITILES], dt.float32, tag="negout")

            for it in range(ITILES):
                rsE8 = accp.tile([128, CH], dt.float32, tag="rsE8")
                rsEM8 = accp.tile([128, CH], dt.float32, tag="rsEM8")
                A8 = accp.tile([128, CH], dt.float32, tag="A8")
                Es = []
                for c in range(CH):
                    S = psum.tile([128, 512], dt.float32, tag="S")
                    for k in range(KT):
                        nc.tensor.matmul(
                            S[:],
                            lhs_sb[k][:, it * 128:(it + 1) * 128],
                            fn_sb[k][c // 2][:, (c % 2) * 512:(c % 2) * 512 + 512],
                            start=(k == 0),
                            stop=(k == KT - 1),
                        )
                    E = ebuf.tile([128, 512], dt.bfloat16, tag="E")
                    nc.scalar.activation(
                        E[:], S[:], AF.Exp, scale=1.0 / TAU,
                        accum_out=rsE8[:, c:c + 1],
                    )
                    s1 = scr.tile([128, 512], dt.bfloat16, tag="scr1")
                    nc.vector.scalar_tensor_tensor(
                        s1[:], tb_sb[:, c * 512:(c + 1) * 512],
                        tcol_sb[:, it:it + 1], E[:],
                        ALU.is_equal, ALU.mult,
                        accum_out=rsEM8[:, c:c + 1],
                    )
                    Es.append(E)

                rsE_t = accp.tile([128, 1], dt.float32, tag="rsE_t")
                rsEM_t = accp.tile([128, 1], dt.float32, tag="rsEM_t")
                neg_t = accp.tile([128, 1], dt.float32, tag="neg_t")
                nc.vector.tensor_reduce(rsE_t[:], rsE8[:], AX.X, ALU.add)
                nc.vector.tensor_reduce(rsEM_t[:], rsEM8[:], AX.X, ALU.add)
                nc.vector.tensor_sub(neg_t[:], rsE_t[:], rsEM_t[:])
                nc.vector.tensor_copy(negout_sb[:, it:it + 1], neg_t[:])

                for c in range(CH):
                    L = lbuf.tile([128, 512], dt.bfloat16, tag="L")
                    nc.scalar.activation(
                        L[:], Es[c][:], AF.Ln, bias=neg_t[:, 0:1], scale=1.0,
                    )
                    s2 = scr.tile([128, 512], dt.bfloat16, tag="scr2")
                    nc.vector.scalar_tensor_tensor(
                        s2[:], tb_sb[:, c * 512:(c + 1) * 512],
                        tcol_sb[:, it:it + 1], L[:],
                        ALU.is_equal, ALU.mult,
                        accum_out=A8[:, c:c + 1],
                    )
                nc.vector.tensor_reduce(aout_sb[:, it:it + 1], A8[:], AX.X, ALU.add)

            nc.sync.dma_start(a_out[:], aout_sb[:])
            nc.sync.dma_start(neg_out[:], negout_sb[:])

    nc.finalize()
    return nc


def _get_nc():
    if "nc" not in _CACHE:
        _CACHE["nc"] = _build_nc()
    return _CACHE["nc"]


def _host_prep(features, targets):
    bf16 = ml_dtypes.bfloat16
    f = np.asarray(features, np.float32)
    t = np.asarray(targets).astype(np.int64)
    rnorm = 1.0 / np.sqrt((f.astype(np.float64) ** 2).sum(1))
    fn = (f * rnorm[:, None].astype(np.float32)).astype(np.float32)
    fnT16 = np.ascontiguousarray(fn.T.astype(bf16))
    t16 = t.astype(np.float32).astype(bf16)
    tb = np.ascontiguousarray(np.broadcast_to(t16[None, :], (128, N)))
    in_maps = []
    for c in range(NCORES):
        sl = slice(c * ROWS, (c + 1) * ROWS)
        in_maps.append({
            "fnT": fnT16,
            "lhsT": np.ascontiguousarray(fnT16[:, sl]),
            "tb": tb,
            "tcol": np.ascontiguousarray(t16[sl].reshape(ITILES, 128).T.astype(np.float32)),
        })
    return fn, t, in_maps


def _host_post(fn, t, a_rows, neg_rows):
    # a_rows/neg_rows: [N] float64, row-ordered
    g = np.zeros((int(t.max()) + 1, D), np.float64)
    np.add.at(g, t, fn.astype(np.float64))
    B = (fn.astype(np.float64) * g[t]).sum(1) / TAU
    corr = np.log(np.exp(1.0 / TAU) + neg_rows) - 1.0 / TAU
    p = np.bincount(t)[t].astype(np.float64)
    numer = a_rows - B - corr
    loss = (numer / p).sum() / p.sum()
    return np.float32(loss)


def _rows_from_out(per_core_outs, key):
    # [128, ITILES] per core, row index = core*512 + it*128 + p
    rows = np.empty(N, np.float64)
    for c, out in enumerate(per_core_outs):
        arr = np.asarray(out[key], np.float64)  # [128, ITILES]
        rows[c * ROWS:(c + 1) * ROWS] = arr.T.reshape(ROWS)
    return rows


def _run(in_maps, trace=False):
    from concourse.bass_utils import run_bass_kernel_spmd
    nc = _get_nc()
    res = run_bass_kernel_spmd(
        nc, in_maps, core_ids=list(range(NCORES)), trace=trace,
    )
    return res


def kernel(features, targets):
    fn, t, in_maps = _host_prep(features, targets)
    res = _run(in_maps, trace=False)
    a_rows = _rows_from_out(res.results, "a_out")
    neg_rows = _rows_from_out(res.results, "neg_out")
    return _host_post(fn, t, a_rows, neg_rows)


# revision 5
# speedup vs baseline: 1.0747x; 1.0747x over previous
"""Supervised-contrastive loss on 8 TRN2 NeuronCores.

Math (matches the reference exactly):
    s_ij   = cosine similarity of feature rows i, j
    E_ij   = exp(s_ij / tau)
    neg_i  = sum_j E_ij * (1 - mask_ij)        (mask = same-class, incl. diag)
    loss   = sum over i and same-class j != i of [ln(E_ij + neg_i) - s_ij/tau] / p_i
             ------------------------------------------------------------
                                  sum_i p_i

Device (per core, rows r in [c*512, (c+1)*512)):
  Phase 1 (exp table set): GEMM S = lhsT.T @ fnT (bf16, f32 PSUM 1024-wide),
    E = exp(S/tau) on ACT with fused row-accumulate (rsE), then one fused
    DVE scalar_tensor_tensor (tb == t_i) * E -> EM tile, row-accumulated
    (rsEM).  neg = rsE - rsEM.
  Phase 2 (ln table set): L = ln(EM + neg_i) via the activation bias, with
    the fused row-accumulator summing ln over the ENTIRE row: masked
    entries contribute ln(E+neg), unmasked ln(neg).
  Outputs per-row lnsum_i and neg_i.  Grouping all exps before all lns
  keeps ACT table-set switches to two.

Host (O(N*D) prep/postprocess only):
    row normalization; A_i = lnsum_i - (N - p_i) * ln(neg_i); the linear
    term B_i = fn_i . g(t_i) / tau via class sums; the diagonal-pair
    correction ln(e^{1/tau} + neg_i) - 1/tau; and the final scalar
    reduction  loss = sum((A - B - corr)/p) / sum(p).
"""

import numpy as np
import ml_dtypes

TAU = 0.1
N, D = 4096, 512
NCORES = 8
ROWS = N // NCORES          # 512 rows per core
ITILES = ROWS // 128        # 4 partition tiles per core
CC = N // 1024              # 4 column chunks of 1024
KT = D // 128               # 4 contraction tiles

_CACHE = {}


def _build_nc():
    import concourse.tile as tile
    import concourse.mybir as mybir
    from concourse import bacc

    dt = mybir.dt
    AF = mybir.ActivationFunctionType
    ALU = mybir.AluOpType
    AX = mybir.AxisListType

    nc = bacc.Bacc(None)
    fnT = nc.declare_dram_parameter("fnT", [D, N], dt.bfloat16, isOutput=False)
    lhsT = nc.declare_dram_parameter("lhsT", [D, ROWS], dt.bfloat16, isOutput=False)
    tb = nc.declare_dram_parameter("tb", [128, N], dt.bfloat16, isOutput=False)
    tcol = nc.declare_dram_parameter("tcol", [128, ITILES], dt.float32, isOutput=False)
    ln_out = nc.declare_dram_parameter("ln_out", [128, ITILES], dt.float32, isOutput=True)
    neg_out = nc.declare_dram_parameter("neg_out", [128, ITILES], dt.float32, isOutput=True)

    with tile.TileContext(nc) as tc:
        with (
            tc.tile_pool(name="persist", bufs=1) as persist,
            tc.tile_pool(name="psum", bufs=4, space="PSUM") as psum,
            tc.tile_pool(name="ebuf", bufs=4) as ebuf,
            tc.tile_pool(name="acc", bufs=2) as accp,
            tc.tile_pool(name="outp", bufs=1) as outp,
        ):
            # ---- persistent loads, ordered so the first GEMM starts early --
            lhs_sb = []
            for k in range(KT):
                tk = persist.tile([128, ROWS], dt.bfloat16, tag=f"lhs_{k}")
                nc.sync.dma_start(tk[:], lhsT[k * 128:(k + 1) * 128, :])
                lhs_sb.append(tk)
            tcol_sb = persist.tile([128, ITILES], dt.float32, tag="tcol")
            nc.sync.dma_start(tcol_sb[:], tcol[:])
            # targets broadcast on the gpsimd DMA queue, in parallel
            tb_sb = persist.tile([128, N], dt.bfloat16, tag="tb")
            for q in range(4):
                nc.gpsimd.dma_start(
                    tb_sb[:, q * 1024:(q + 1) * 1024],
                    tb[:, q * 1024:(q + 1) * 1024],
                )
            # fnT quarter-by-quarter in consumption order (all kt of q first)
            fn_sb = [[None] * CC for _ in range(KT)]
            for q in range(CC):
                for k in range(KT):
                    tq = persist.tile([128, 1024], dt.bfloat16, tag=f"fnt_{k}_{q}")
                    nc.sync.dma_start(
                        tq[:], fnT[k * 128:(k + 1) * 128, q * 1024:(q + 1) * 1024]
                    )
                    fn_sb[k][q] = tq

            lnout_sb = outp.tile([128, ITILES], dt.float32, tag="lnout")
            negout_sb = outp.tile([128, ITILES], dt.float32, tag="negout")

            # ---- phase 1: GEMM + exp + masked row sums ----
            EMs = []   # [it][cc] -> [128, 1024] bf16, E*mask (kept for phase 2)
            negs = []  # [it] -> [128, 1] f32
            for it in range(ITILES):
                rsE4 = accp.tile([128, CC], dt.float32, tag="rsE4")
                rsEM4 = accp.tile([128, CC], dt.float32, tag="rsEM4")
                em_t = []
                for cc in range(CC):
                    S = psum.tile([128, 1024], dt.float32, tag="S")
                    for h in range(2):
                        for k in range(KT):
                            nc.tensor.matmul(
                                S[:, h * 512:(h + 1) * 512],
                                lhs_sb[k][:, it * 128:(it + 1) * 128],
                                fn_sb[k][cc][:, h * 512:(h + 1) * 512],
                                start=(k == 0),
                                stop=(k == KT - 1),
                            )
                    E = ebuf.tile([128, 1024], dt.bfloat16, tag="E")
                    nc.scalar.activation(
                        E[:], S[:], AF.Exp, scale=1.0 / TAU,
                        accum_out=rsE4[:, cc:cc + 1],
                    )
                    EM = persist.tile([128, 1024], dt.bfloat16, tag=f"em_{it}_{cc}")
                    nc.vector.scalar_tensor_tensor(
                        EM[:], tb_sb[:, cc * 1024:(cc + 1) * 1024],
                        tcol_sb[:, it:it + 1], E[:],
                        ALU.is_equal, ALU.mult,
                        accum_out=rsEM4[:, cc:cc + 1],
                    )
                    em_t.append(EM)
                EMs.append(em_t)

                rsE_t = accp.tile([128, 1], dt.float32, tag="rsE_t")
                rsEM_t = accp.tile([128, 1], dt.float32, tag="rsEM_t")
                neg_t = accp.tile([128, 1], dt.float32, tag=f"neg_{it}")
                nc.vector.tensor_reduce(rsE_t[:], rsE4[:], AX.X, ALU.add)
                nc.vector.tensor_reduce(rsEM_t[:], rsEM4[:], AX.X, ALU.add)
                nc.vector.tensor_sub(neg_t[:], rsE_t[:], rsEM_t[:])
                nc.vector.tensor_copy(negout_sb[:, it:it + 1], neg_t[:])
                negs.append(neg_t)

            # ---- phase 2: full-row ln(EM + neg) accumulation ----
            for it in range(ITILES):
                ln4 = accp.tile([128, CC], dt.float32, tag=f"ln4_{it}")
                for cc in range(CC):
                    L = ebuf.tile([128, 1024], dt.bfloat16, tag="L")
                    nc.scalar.activation(
                        L[:], EMs[it][cc][:], AF.Ln,
                        bias=negs[it][:, 0:1], scale=1.0,
                        accum_out=ln4[:, cc:cc + 1],
                    )
                nc.vector.tensor_reduce(lnout_sb[:, it:it + 1], ln4[:], AX.X, ALU.add)

            nc.sync.dma_start(ln_out[:], lnout_sb[:])
            nc.sync.dma_start(neg_out[:], negout_sb[:])

    nc.finalize()
    return nc


def _get_nc():
    if "nc" not in _CACHE:
        _CACHE["nc"] = _build_nc()
    return _CACHE["nc"]


def _host_prep(features, targets):
    bf16 = ml_dtypes.bfloat16
    f = np.asarray(features, np.float32)
    t = np.asarray(targets).astype(np.int64)
    rnorm = 1.0 / np.sqrt((f.astype(np.float64) ** 2).sum(1))
    fn = (f * rnorm[:, None].astype(np.float32)).astype(np.float32)
    fnT16 = np.ascontiguousarray(fn.T.astype(bf16))
    t16 = t.astype(np.float32).astype(bf16)
    tb = np.ascontiguousarray(np.broadcast_to(t16[None, :], (128, N)))
    in_maps = []
    for c in range(NCORES):
        sl = slice(c * ROWS, (c + 1) * ROWS)
        in_maps.append({
            "fnT": fnT16,
            "lhsT": np.ascontiguousarray(fnT16[:, sl]),
            "tb": tb,
            "tcol": np.ascontiguousarray(t16[sl].reshape(ITILES, 128).T.astype(np.float32)),
        })
    return fn, t, in_maps


def _host_post(fn, t, lnsum_rows, neg_rows):
    # lnsum_rows/neg_rows: [N] float64, row-ordered
    p = np.bincount(t)[t].astype(np.float64)
    A = lnsum_rows - (N - p) * np.log(neg_rows)
    g = np.zeros((int(t.max()) + 1, D), np.float64)
    np.add.at(g, t, fn.astype(np.float64))
    B = (fn.astype(np.float64) * g[t]).sum(1) / TAU
    corr = np.log(np.exp(1.0 / TAU) + neg_rows) - 1.0 / TAU
    numer = A - B - corr
    loss = (numer / p).sum() / p.sum()
    return np.float32(loss)


def _rows_from_out(per_core_outs, key):
    # [128, ITILES] per core, row index = core*512 + it*128 + p
    rows = np.empty(N, np.float64)
    for c, out in enumerate(per_core_outs):
        arr = np.asarray(out[key], np.float64)  # [128, ITILES]
        rows[c * ROWS:(c + 1) * ROWS] = arr.T.reshape(ROWS)
    return rows


def _run(in_maps, trace=False):
    from concourse.bass_utils import run_bass_kernel_spmd
    nc = _get_nc()
    res = run_bass_kernel_spmd(
        nc, in_maps, core_ids=list(range(NCORES)), trace=trace,
    )
    return res


def kernel(features, targets):
    fn, t, in_maps = _host_prep(features, targets)
    res = _run(in_maps, trace=False)
    lnsum_rows = _rows_from_out(res.results, "ln_out")
    neg_rows = _rows_from_out(res.results, "neg_out")
    return _host_post(fn, t, lnsum_rows, neg_rows)
